# revision 1
# baseline (speedup 1.0000x reference)
"""Contrastive loss (SimCLR-style) on 8 Trainium2 NeuronCores.

Full inputs in, full output out.  Each core owns a 1024-row block of
feats; the host passes each core a rolled, pre-tiled bf16 copy of feats
so the block is always local rows 0..1023 (static self-mask diagonal,
identical SPMD program on every core) and every DMA line is contiguous.

Symmetry split: exp(cos/T) is symmetric, so core x only computes its
block rows against local column blocks 0..4 (cols 0..5119).  Row sums
over the remaining column blocks 5..7 are recovered from *column* sums
of blocks (x, x+1..x+3), which other cores' rows need by symmetry:
column sums are accumulated on the PE with a ones-stationary matmul and
shipped to the host, which adds them into the right rows.

Engine budget per core: ACT does only Exp (one table load for the whole
kernel; rsqrt for row norms is a Quake-style integer Newton iteration on
the DVE).  Pool computes the row sum-of-squares (fused square+row-sum
scalar_tensor_tensor) and half the PSUM->SBUF nfT copies; DVE does the
normalize scaling, the self-mask add, and the other half of the copies;
PE does bf16 transposes, the similarity matmuls and the exp column-sum
matmuls.  Host: assemble S, logsumexp, mean.
"""

from contextlib import ExitStack

import numpy as np

N, D, NCORES = 8192, 128, 8
BLK = N // NCORES            # 1024 rows per core
TPB = BLK // 128             # 8 M-tiles (of 128 rows) per core
NT = N // 128                # 64 row tiles total
NTU = 40                     # tiles actually used on device (cols 0..5119)
TEMP = 0.07
EPS = 1e-8
MASK_SUB = 30.0              # cos - 30 -> exp((cos-30)/T) == 0 in fp32
CHUNK = 512                  # matmul moving-operand columns (1 PSUM bank out)
QCOLS = 1024                 # psum tile columns (2 banks)
NQ = 5                       # direct column blocks per core (cols 0..5119)
CSBLKS = 3                   # column-sum blocks (local col blocks 1..3)
MAGIC = 0x5F3759DF           # Quake rsqrt seed

_CACHE = {}
LAST_RESULT = None


def _rsqrt_cols(nc, mybir, ss, ti, t2, rall, cols):
    """rall[:, cols] = 1/sqrt(max(ss[:, cols], EPS^2)), DVE only.

    Quake bit hack (C - (i>>1) done as (i>>1 xor -1) + (C+1)) plus three
    Newton steps: y *= 1.5 - 0.5*ss*y*y.  ~2.7e-7 max rel error.
    """
    ALU = mybir.AluOpType
    f32 = mybir.dt.float32
    i32 = mybir.dt.int32
    sc = ss[:, cols]
    nc.vector.tensor_scalar_max(sc, sc, EPS * EPS)
    nc.vector.tensor_scalar(ti[:, cols], sc.bitcast(i32), 1, -1,
                            ALU.logical_shift_right, ALU.bitwise_xor)
    nc.vector.tensor_scalar(ti[:, cols], ti[:, cols], MAGIC + 1, None,
                            ALU.add)
    y = rall[:, cols]
    nc.vector.tensor_copy(y, ti[:, cols].bitcast(f32))
    for _ in range(3):
        nc.vector.tensor_mul(t2[:, cols], y, y)
        nc.vector.tensor_mul(t2[:, cols], t2[:, cols], sc)
        nc.vector.tensor_scalar(t2[:, cols], t2[:, cols], -0.5, 1.5,
                                ALU.mult, ALU.add)
        nc.vector.tensor_mul(y, y, t2[:, cols])


def _emit(tc, xr, pr, idb, eyeneg_d, s_out, pos_out, cs_out, rep=0):
    import concourse.mybir as mybir

    nc = tc.nc
    f32 = mybir.dt.float32
    bf16 = mybir.dt.bfloat16
    i32 = mybir.dt.int32
    AF = mybir.ActivationFunctionType
    AX = mybir.AxisListType.X
    ALU = mybir.AluOpType
    NA = TPB            # tiles per norm chunk
    NCOL = NTU + TPB    # ss columns: 40 used row tiles + 8 partner tiles

    with ExitStack() as ctx:
        singles = ctx.enter_context(tc.tile_pool(name=f"singles{rep}", bufs=1))
        work = ctx.enter_context(tc.tile_pool(name=f"work{rep}", bufs=8))

        xbig = singles.tile([128, NTU * D], bf16, tag="xbig")    # rolled X, tiled
        nfT = singles.tile([128, NTU * D], bf16, tag="nfT")      # normalized X^T
        nfblk = singles.tile([128, BLK], bf16, tag="nfblk")      # nf rows 0..1023
        pbig = singles.tile([128, TPB * D], bf16, tag="pbig")    # partner rows
        ss = singles.tile([128, NCOL], f32, tag="ss")
        ti = singles.tile([128, NCOL], i32, tag="ti")
        t2 = singles.tile([128, NCOL], f32, tag="t2")
        rall = singles.tile([128, NCOL], f32, tag="rall")
        posv = singles.tile([128, TPB], f32, tag="posv")
        parts = singles.tile([128, TPB * NQ], f32, tag="parts")
        identb = singles.tile([128, 128], bf16, tag="identb")
        eyeneg = singles.tile([128, 128], bf16, tag="eyeneg")
        onesb = singles.tile([128, 128], bf16, tag="onesb")
        colacc = singles.tile([128, CSBLKS * QCOLS], f32, tag="colacc")

        # ---- loads (contiguous per-partition lines; host pre-tiled) ----
        # local block's tiles first so the norm pipeline starts immediately;
        # the rest in one large transfer (each dma_start costs ~650ns of SP
        # issue time, and one queue already saturates HBM bandwidth)
        HC = NA // 2
        nc.sync.dma_start(out=xbig[:, 0:HC * D], in_=xr[:, 0:HC * D])
        nc.sync.dma_start(out=xbig[:, HC * D:NA * D], in_=xr[:, HC * D:NA * D])
        nc.sync.dma_start(out=identb[:], in_=idb)
        nc.sync.dma_start(out=eyeneg[:], in_=eyeneg_d)
        nc.sync.dma_start(out=xbig[:, NA * D:NTU * D], in_=xr[:, NA * D:NTU * D])
        nc.sync.dma_start(out=pbig[:], in_=pr)
        nc.vector.memset(onesb[:], 1.0)

        # prime the ACT Exp table at t~0 so the first real exp doesn't pay
        # the table load on the critical path
        warm = singles.tile([128, 1], f32, tag="warm")
        nc.vector.memset(warm[:], 0.0)
        nc.scalar.activation(warm[:], warm[:], AF.Exp)

        def emit_ss(t, eng=None):
            # stt+accum is DVE-only in walrus codegen
            j = work.tile([128, D], bf16, tag="junk")
            nc.vector.scalar_tensor_tensor(j[:], xbig[:, t * D:(t + 1) * D], 1.0,
                                           xbig[:, t * D:(t + 1) * D],
                                           ALU.mult, ALU.mult,
                                           accum_out=ss[:, t:t + 1])

        def emit_ssp(t):
            j = work.tile([128, D], bf16, tag="junk")
            nc.vector.scalar_tensor_tensor(j[:], pbig[:, t * D:(t + 1) * D], 1.0,
                                           pbig[:, t * D:(t + 1) * D],
                                           ALU.mult, ALU.mult,
                                           accum_out=ss[:, NTU + t:NTU + t + 1])

        # ---- interleaved pipeline: chunk-k norms feed the q=k round ----
        # Per chunk k (8 row tiles): Pool sum-of-squares, DVE rsqrt+scale,
        # PE transpose, Pool/DVE PSUM->SBUF copy; then the q=k similarity
        # round (matmuls + exp row-sums + column sums) consumes those nfT
        # columns while chunk k+1's norms run on the other engines.
        with (
            tc.tile_pool(name=f"tpsum{rep}", bufs=2, space="PSUM") as tpsum,
            tc.tile_pool(name=f"mpsum{rep}", bufs=2, space="PSUM") as mpsum,
            tc.tile_pool(name=f"cpsum{rep}", bufs=1, space="PSUM") as cpsum,
            tc.tile_pool(name=f"escratch{rep}", bufs=6) as esp,
        ):
            def emit_nft(t, keep_nf):
                # Pool can scale (SBUF->SBUF) but cannot read PSUM, so the
                # copy-back always rides DVE; chunk 0 stays all-DVE (latency)
                seng = nc.vector if t < NA else nc.gpsimd
                if keep_nf:
                    nf_ap = nfblk[:, t * D:(t + 1) * D]
                else:
                    nf_t = work.tile([128, D], bf16, tag="nf")
                    nf_ap = nf_t[:]
                seng.tensor_scalar_mul(nf_ap, xbig[:, t * D:(t + 1) * D],
                                       rall[:, t:t + 1])
                pt = tpsum.tile([128, 128], bf16, tag="tp")
                nc.tensor.transpose(pt[:], nf_ap, identb[:])
                nc.vector.tensor_copy(nfT[:, t * D:(t + 1) * D], pt[:])

            def emit_chunk(k, halves=1):
                # chunk 0 is the whole startup critical path: keep it on DVE
                # end-to-end (no cross-engine semaphore hops)
                eng = nc.vector if k == 0 else None
                t0 = k * NA
                step = NA // halves
                for h in range(halves):
                    h0 = t0 + h * step
                    for t in range(h0, h0 + step):
                        emit_ss(t, eng)
                    _rsqrt_cols(nc, mybir, ss, ti, t2, rall, slice(h0, h0 + step))
                    for t in range(h0, h0 + step):
                        emit_nft(t, keep_nf=(k == 0))

            def emit_qround(q):
                do_cs = 1 <= q <= CSBLKS
                if do_cs:
                    cs0 = cpsum.tile([128, CHUNK], f32, tag="cs0")
                    cs1 = cpsum.tile([128, CHUNK], f32, tag="cs1")
                for m in range(TPB):
                    lhsT = nfT[:, m * 128:(m + 1) * 128]
                    pt = mpsum.tile([128, QCOLS], f32, tag="mp")
                    for jj in range(QCOLS // CHUNK):
                        n0 = q * QCOLS + jj * CHUNK
                        diag = q == 0 and jj == (m * 128) // CHUNK
                        nc.tensor.matmul(
                            pt[:, jj * CHUNK:(jj + 1) * CHUNK],
                            lhsT, nfT[:, n0:n0 + CHUNK], start=True,
                            stop=not diag, skip_group_check=diag,
                        )
                        if diag:
                            # self column of local row m*128+p is m*128+p:
                            # accumulate -MASK_SUB*I via a second matmul so the
                            # mask never rides the DVE queue
                            c0 = m * 128 - jj * CHUNK
                            nc.tensor.matmul(
                                pt[:, jj * CHUNK + c0:jj * CHUNK + c0 + 128],
                                identb[:], eyeneg[:], start=False, stop=True,
                                skip_group_check=True,
                            )
                    e = esp.tile([128, QCOLS], bf16, tag="e")
                    nc.scalar.activation(
                        e[:], pt[:], AF.Exp, scale=1.0 / TEMP,
                        accum_out=parts[:, m * NQ + q:m * NQ + q + 1],
                    )
                    if q == NQ - 1:
                        # last round: this m's parts row is final; ship it
                        nc.sync.dma_start(
                            out=s_out[:, m * NQ:(m + 1) * NQ],
                            in_=parts[:, m * NQ:(m + 1) * NQ])
                    if do_cs:
                        # column sums of exp accumulated across the 8 M-tiles
                        nc.tensor.matmul(cs0[:], onesb[:], e[:, :CHUNK],
                                         start=(m == 0), stop=(m == TPB - 1),
                                         skip_group_check=True)
                        nc.tensor.matmul(cs1[:], onesb[:], e[:, CHUNK:],
                                         start=(m == 0), stop=(m == TPB - 1),
                                         skip_group_check=True)
                if do_cs:
                    k = q - 1
                    nc.vector.tensor_copy(colacc[0:1, k * QCOLS:k * QCOLS + CHUNK],
                                          cs0[0:1, :])
                    nc.vector.tensor_copy(colacc[0:1, k * QCOLS + CHUNK:(k + 1) * QCOLS],
                                          cs1[0:1, :])

            def emit_phase_b():
                # positive-pair cosines; runs in Pool/DVE idle slots mid-kernel
                for t in range(TPB):
                    emit_ssp(t)
                _rsqrt_cols(nc, mybir, ss, ti, t2, rall, slice(NTU, NCOL))
                for t in range(TPB):
                    npf = work.tile([128, D], bf16, tag="nf")
                    nc.gpsimd.tensor_scalar_mul(npf[:], pbig[:, t * D:(t + 1) * D],
                                                rall[:, NTU + t:NTU + t + 1])
                    j = work.tile([128, D], bf16, tag="junk")
                    nc.vector.scalar_tensor_tensor(j[:], nfblk[:, t * D:(t + 1) * D],
                                                   1.0, npf[:], ALU.mult, ALU.mult,
                                                   accum_out=posv[:, t:t + 1])
                nc.sync.dma_start(out=pos_out, in_=posv[:])

            with tc.high_priority():
                emit_chunk(0, halves=2)
            for q in range(NQ):
                with tc.high_priority():
                    emit_qround(q)
                if q + 1 < NTU // NA:
                    emit_chunk(q + 1)
                if q == 3:
                    emit_phase_b()

        nc.sync.dma_start(out=cs_out, in_=colacc[0:1, :])


def declare_io(nc):
    """Declare the kernel's DRAM I/O on `nc`; returns the APs _emit wants."""
    import concourse.mybir as mybir

    f32 = mybir.dt.float32
    bf16 = mybir.dt.bfloat16
    xr_h = nc.dram_tensor("xr", [128, NTU * D], bf16, kind="ExternalInput")
    pr_h = nc.dram_tensor("partner", [128, TPB * D], bf16, kind="ExternalInput")
    id_h = nc.dram_tensor("identb", [128, 128], bf16, kind="ExternalInput")
    en_h = nc.dram_tensor("eyeneg", [128, 128], bf16, kind="ExternalInput")
    s_h = nc.dram_tensor("s_out", [128, TPB * NQ], f32, kind="ExternalOutput")
    p_h = nc.dram_tensor("pos_out", [128, TPB], f32, kind="ExternalOutput")
    c_h = nc.dram_tensor("cs_out", [1, CSBLKS * QCOLS], f32, kind="ExternalOutput")
    return (xr_h.ap(), pr_h.ap(), id_h.ap(), en_h.ap(),
            s_h.ap(), p_h.ap(), c_h.ap())


def _build_nc(repeats=1):
    import concourse.tile as tile
    from concourse import bacc

    nc = bacc.Bacc(
        "TRN2", target_bir_lowering=False, debug=False,
        enable_asserts=False, num_devices=NCORES,
    )
    aps = declare_io(nc)
    with tile.TileContext(nc, trace_sim=False) as tc:
        for rep in range(repeats):
            _emit(tc, *aps, rep=rep)
    nc.compile()
    return nc


def get_nc(repeats=1):
    key = ("nc", repeats)
    if key not in _CACHE:
        _CACHE[key] = _build_nc(repeats)
    return _CACHE[key]


def _tiled(a):
    """[T*128, D] row-major -> [128, T*D] where partition p holds rows
    p, 128+p, ... as contiguous D-blocks (device reads straight lines)."""
    t = a.shape[0] // 128
    return np.ascontiguousarray(
        a.reshape(t, 128, D).transpose(1, 0, 2).reshape(128, t * D))


def make_in_maps(feats, label):
    import ml_dtypes

    feats = np.asarray(feats, dtype=np.float32)
    label = np.asarray(label)
    pos_idx = np.argmax(label, axis=1)
    fb = feats.astype(ml_dtypes.bfloat16)
    partner = fb[pos_idx]
    identb = np.eye(128).astype(ml_dtypes.bfloat16)
    eyeneg = (-MASK_SUB * np.eye(128)).astype(ml_dtypes.bfloat16)
    in_maps = []
    for c in range(NCORES):
        xr = np.concatenate([fb[c * BLK:], fb[:c * BLK]], axis=0)[:NTU * 128]
        in_maps.append({
            "xr": _tiled(xr),
            "partner": _tiled(partner[c * BLK:(c + 1) * BLK]),
            "identb": identb,
            "eyeneg": eyeneg,
        })
    return in_maps


def finish(results):
    """Host epilogue: assemble full row sums from direct row partials and
    symmetric column partials, then logsumexp and mean."""
    S = np.zeros(N, dtype=np.float64)
    pos = np.zeros(N, dtype=np.float64)
    for x in range(NCORES):
        pv_ = results[x]["s_out"].astype(np.float64)      # [128, TPB*NQ]
        sv = pv_.reshape(128, TPB, NQ).sum(axis=2)        # [128, TPB]
        S[x * BLK:(x + 1) * BLK] += sv.T.reshape(-1)      # local rows in order
        pv = results[x]["pos_out"].astype(np.float64)
        pos[x * BLK:(x + 1) * BLK] = pv.T.reshape(-1)
        cs = results[x]["cs_out"].astype(np.float64).reshape(CSBLKS, BLK)
        for k in range(1, CSBLKS + 1):
            tgt = ((x + k) % NCORES) * BLK                # rows of block x+k
            S[tgt:tgt + BLK] += cs[k - 1]
    lse = np.log(S)
    loss = (lse - pos / TEMP).mean()
    return np.array(loss, dtype=np.float32)


def kernel(feats, label, _trace=False, _repeats=1):
    global LAST_RESULT
    from concourse.bass_utils import run_bass_kernel_spmd

    nc = get_nc(_repeats)
    in_maps = make_in_maps(feats, label)
    res = run_bass_kernel_spmd(nc, in_maps, list(range(NCORES)), trace=_trace)
    LAST_RESULT = res
    return finish(res.results)



# revision 2
# speedup vs baseline: 2.4425x; 2.4425x over previous
"""Contrastive loss (SimCLR-style) on 8 Trainium2 NeuronCores.

Full inputs in, full output out.  Each core owns a 1024-row block of
feats.  The host pre-normalizes feats (fp32), casts to bf16, and passes
each core the TRANSPOSED layout nfT = nf.T [D=128, cols], rolled so the
core's own block is columns 0..1023.  Because cos = nfT.T @ nfT, the
transposed layout serves as BOTH matmul operands: no on-device norms,
no transposes, no rsqrt pipeline.

Symmetry split: exp(cos/T) is symmetric, so core x only computes its
1024 rows against local column blocks 0..4 (cols 0..5119).  Row sums
over the remaining column blocks 5..7 are recovered from *column* sums
of blocks 1..3 (ones-stationary matmuls accumulated over the 8 row
tiles), shipped to the host, which adds them into the right rows.

The self-similarity diagonal is NOT masked on device: the host knows
exactly what the device computes for z_ii (fp32 dot of the bf16 row
with itself) and subtracts exp(z_ii/T) from the assembled row sums.

Engine budget per core: PE does 80 similarity matmuls (5 blocks x 8
m-tiles x 2x512) plus 48 column-sum matmuls; ACT does all 40 exp ops
([128,1024] PSUM->SBUF, free row-sum accumulation); DVE only copies the
3 column-sum rows out of PSUM.  Host: normalize, transpose, positive
pairs, diag subtraction, logsumexp, mean.
"""

from contextlib import ExitStack

import numpy as np

N, D, NCORES = 8192, 128, 8
BLK = N // NCORES            # 1024 rows per core
TPB = BLK // 128             # 8 M-tiles (of 128 rows) per core
NQ = 5                       # direct column blocks per core (cols 0..5119)
CSBLKS = 3                   # column-sum blocks (local col blocks 1..3)
COLS = NQ * BLK              # 5120 columns held on device
TEMP = 0.07
EPS = 1e-8

_CACHE = {}
_AUX = {}
LAST_RESULT = None


def _emit(tc, xt, s_out, cs_out, rep=0):
    import concourse.mybir as mybir

    nc = tc.nc
    f32 = mybir.dt.float32
    bf16 = mybir.dt.bfloat16
    AF = mybir.ActivationFunctionType
    SCALE = 1.0 / TEMP

    with ExitStack() as ctx:
        singles = ctx.enter_context(tc.tile_pool(name=f"singles{rep}", bufs=1))
        esp = ctx.enter_context(tc.tile_pool(name=f"esp{rep}", bufs=TPB))

        xbig = singles.tile([128, COLS], bf16, tag="xbig")    # rolled nf^T
        onesb = singles.tile([128, 128], bf16, tag="onesb")
        parts = singles.tile([128, NQ * TPB], f32, tag="parts")
        colacc = singles.tile([1, CSBLKS * BLK], f32, tag="colacc")

        # block 0 first (stationaries + q=0 moving), rest streams in under
        # the q=0 round's compute
        nc.sync.dma_start(out=xbig[:, 0:BLK], in_=xt[:, 0:BLK])
        nc.sync.dma_start(out=xbig[:, BLK:COLS], in_=xt[:, BLK:COLS])
        nc.vector.memset(onesb[:], 1.0)

        # prime the ACT Exp table at t~0 so the first real exp doesn't pay
        # the table load on the critical path
        warm = singles.tile([128, 1], f32, tag="warm")
        nc.vector.memset(warm[:], 0.0)
        nc.scalar.activation(warm[:], warm[:], AF.Exp)

        with (
            tc.tile_pool(name=f"mpsum{rep}", bufs=3, space="PSUM") as mpsum,
            tc.tile_pool(name=f"cpsum{rep}", bufs=1, space="PSUM") as cpsum,
        ):
            for q in range(NQ):
                do_cs = 1 <= q <= CSBLKS
                etiles = []
                for m in range(TPB):
                    z = mpsum.tile([128, BLK], f32, tag="z")
                    lhsT = xbig[:, m * 128:(m + 1) * 128]
                    c0 = q * BLK
                    nc.tensor.matmul(z[:, 0:512], lhsT, xbig[:, c0:c0 + 512])
                    nc.tensor.matmul(z[:, 512:1024], lhsT,
                                     xbig[:, c0 + 512:c0 + 1024])
                    e = esp.tile([128, BLK], bf16, tag="e")
                    col = q * TPB + m
                    nc.scalar.activation(e[:], z[:], AF.Exp, scale=SCALE,
                                         accum_out=parts[:, col:col + 1])
                    etiles.append(e)
                if do_cs:
                    # column sums of exp accumulated across the 8 M-tiles;
                    # one ones-LDWEIGHTS per round (batched after the e's)
                    cs = cpsum.tile([128, BLK], f32, tag="cs")
                    for m in range(TPB):
                        nc.tensor.matmul(cs[:, 0:512], onesb[:],
                                         etiles[m][:, 0:512],
                                         start=(m == 0), stop=(m == TPB - 1),
                                         skip_group_check=True)
                        nc.tensor.matmul(cs[:, 512:1024], onesb[:],
                                         etiles[m][:, 512:1024],
                                         start=(m == 0), stop=(m == TPB - 1),
                                         skip_group_check=True)
                    k = q - 1
                    nc.vector.tensor_copy(colacc[0:1, k * BLK:(k + 1) * BLK],
                                          cs[0:1, :])

        nc.sync.dma_start(out=s_out, in_=parts[:])
        nc.sync.dma_start(out=cs_out, in_=colacc[:])


def declare_io(nc):
    """Declare the kernel's DRAM I/O on `nc`; returns the APs _emit wants."""
    import concourse.mybir as mybir

    f32 = mybir.dt.float32
    bf16 = mybir.dt.bfloat16
    xt_h = nc.dram_tensor("xt", [128, COLS], bf16, kind="ExternalInput")
    s_h = nc.dram_tensor("s_out", [128, NQ * TPB], f32, kind="ExternalOutput")
    c_h = nc.dram_tensor("cs_out", [1, CSBLKS * BLK], f32,
                         kind="ExternalOutput")
    return (xt_h.ap(), s_h.ap(), c_h.ap())


def _build_nc(repeats=1):
    import concourse.tile as tile
    from concourse import bacc

    nc = bacc.Bacc(
        "TRN2", target_bir_lowering=False, debug=False,
        enable_asserts=False, num_devices=NCORES,
    )
    aps = declare_io(nc)
    with tile.TileContext(nc, trace_sim=False) as tc:
        for rep in range(repeats):
            _emit(tc, *aps, rep=rep)
    nc.compile()
    return nc


def get_nc(repeats=1):
    key = ("nc", repeats)
    if key not in _CACHE:
        _CACHE[key] = _build_nc(repeats)
    return _CACHE[key]


def make_in_maps(feats, label):
    import ml_dtypes

    feats = np.asarray(feats, dtype=np.float32)
    label = np.asarray(label)
    norms = np.sqrt((feats ** 2).sum(axis=1))
    nf = feats / np.maximum(norms, EPS)[:, None]
    nf16 = nf.astype(ml_dtypes.bfloat16)
    nfT = np.ascontiguousarray(nf16.T)                 # [128, 8192]
    nfT2 = np.concatenate([nfT, nfT], axis=1)          # wraparound roll
    in_maps = [
        {"xt": np.ascontiguousarray(nfT2[:, c * BLK:c * BLK + COLS])}
        for c in range(NCORES)
    ]

    # host-side replication of the device's self-similarity term:
    # z_ii = fp32 dot of the bf16 row with itself, e_ii = exp(z_ii * 1/T)
    nf16f = nf16.astype(np.float32)
    ssq = (nf16f ** 2).sum(axis=1, dtype=np.float32)
    _AUX["diag"] = np.exp((ssq * np.float32(1.0 / TEMP)).astype(np.float32)
                          ).astype(np.float64)
    pos_idx = np.argmax(label, axis=1)
    nf64 = nf.astype(np.float64)
    _AUX["pos"] = (nf64 * nf64[pos_idx]).sum(axis=1) / TEMP
    return in_maps


def finish(results):
    """Host epilogue: assemble full row sums from direct row partials and
    symmetric column partials, subtract the exact diagonal, logsumexp,
    subtract positive-pair logits, mean."""
    S = np.zeros(N, dtype=np.float64)
    for x in range(NCORES):
        parts = results[x]["s_out"].astype(np.float64)     # [128, NQ*TPB]
        sv = parts.reshape(128, NQ, TPB).sum(axis=1)       # [128, TPB]
        S[x * BLK:(x + 1) * BLK] += sv.T.reshape(-1)       # row = m*128+p
        cs = results[x]["cs_out"].astype(np.float64).reshape(CSBLKS, BLK)
        for k in range(1, CSBLKS + 1):
            tgt = ((x + k) % NCORES) * BLK                 # rows of block x+k
            S[tgt:tgt + BLK] += cs[k - 1]
    S -= _AUX["diag"]
    lse = np.log(S)
    loss = (lse - _AUX["pos"]).mean()
    return np.array(loss, dtype=np.float32)


def kernel(feats, label, _trace=False, _repeats=1):
    global LAST_RESULT
    from concourse.bass_utils import run_bass_kernel_spmd

    nc = get_nc(_repeats)
    in_maps = make_in_maps(feats, label)
    res = run_bass_kernel_spmd(nc, in_maps, list(range(NCORES)), trace=_trace)
    LAST_RESULT = res
    return finish(res.results)
